# revision 10
# baseline (speedup 1.0000x reference)
"""KAN layer (Chebyshev deg-8) Trainium2 kernel, 8-core data-parallel.

Math: out[b] = sum_n hw[n] * (X @ C.T)[b,n] = X[b,:] @ (C.T @ hw)
            = sum_d g_d(tanh x[d,b]),   g_d = sum_k W[d,k] T_k(u)

Reparametrize per-dim in t = T2(u) = 2u^2-1, s = t^2 (both in [-1,1], well
conditioned in bf16):
  g = a0 + a1 t + a2 s + a3 st + a4 s^2 + u(b0 + b1 t + b2 s + b3 st)
Pair the 8 non-constant coefficients into 4 streams via the ratio trick,
each one fused scalar_tensor_tensor op  w*(in0 + c)*in1:
  E1 = (s + a1/a3)*t  (w=a3)   E2 = (s + a2/a4)*s  (w=a4)
  Z1 = (t + b0/b1)*u  (w=b1)   Z2 = (t + b2/b3)*us (w=b3)
Engine split per [128,BLK] tile: ACT{tanh->f32, q=Sq(u), s=Sq(t)},
DVE{t=2q-1, E1, E2, Z1, Z2}, Pool{us=u*s}, PE{4 streams -> PSUM},
DMA{x in, PSUM -> y out}.  a0-sum is added on host.
"""
import sys
import numpy as np

sys.path.insert(0, "/opt/trn_rl_repo")

import orjson
from contextlib import ExitStack

import concourse.bass as bass
from concourse import mybir
from concourse.tile import TileContext
from concourse.bass_utils import run_bass_kernel_spmd

F32 = mybir.dt.float32
BF16 = mybir.dt.bfloat16
AF = mybir.ActivationFunctionType
OP = mybir.AluOpType

B, D, DEG1 = 32768, 256, 9
NCORES = 8
BC = B // NCORES          # 4096 batch per core
NCH = D // 128            # 2 partition chunks of dims
NS = 4                    # streams per chunk
BLK = 2048                # free-dim block for pipelining
NBLK = BC // BLK

# ---- walrus workaround: split >1 sem-waits onto Drain carriers -------------
_MAXW = 1

def _split_waits(bir_json: bytes) -> bytes:
    d = orjson.loads(bir_json)
    for fn in d.get("functions", []):
        for bb in fn.get("blocks", []):
            out = []
            for ins in bb.get("instructions", []):
                si = ins.get("sync_info") or {}
                waits = si.get("on_wait") or []
                if len(waits) > _MAXW:
                    extra, keep = waits[:-_MAXW], waits[-_MAXW:]
                    for i in range(0, len(extra), _MAXW):
                        out.append({
                            "debug": ins.get("debug", 0),
                            "engine": ins["engine"], "ins": [], "outs": [],
                            "name": f"{ins['name']}_ws{i}", "opcode": "Drain",
                            "sync_info": {"on_update": [],
                                          "on_wait": extra[i:i + _MAXW]},
                        })
                    si["on_wait"] = keep
                out.append(ins)
            bb["instructions"] = out
    return orjson.dumps(d)

def _install_patch():
    import concourse.bass_utils as bu
    if getattr(bu, "_ws_patched", False):
        return
    orig = bu.compile_bir_kernel
    def patched(bir_json, tmpdir, neff_name="file.neff"):
        return orig(_split_waits(bir_json), tmpdir, neff_name)
    bu.compile_bir_kernel = patched
    bu._ws_patched = True
    try:
        import concourse.bass2jax as b2j
        if getattr(b2j, "compile_bir_kernel", None) is orig:
            b2j.compile_bir_kernel = patched
    except Exception:
        pass

# ---- device kernel ---------------------------------------------------------
def _build():
    nc = bass.Bass()
    xt = nc.declare_dram_parameter("xt", [D, BC], F32, isOutput=False)
    wv = nc.declare_dram_parameter("wv", [128, NCH * NS], F32, isOutput=False)
    cv = nc.declare_dram_parameter("cv", [128, NCH * NS], F32, isOutput=False)
    y = nc.declare_dram_parameter("y", [1, BC], F32, isOutput=True)

    with TileContext(nc) as tc, ExitStack() as ctx:
        cpool = ctx.enter_context(tc.tile_pool(name="const", bufs=1))
        xp = ctx.enter_context(tc.tile_pool(name="xin", bufs=3))
        fp = ctx.enter_context(tc.tile_pool(name="feat", bufs=3))
        pp = ctx.enter_context(tc.tile_pool(name="ps", bufs=2, space="PSUM"))

        wf = cpool.tile([128, NCH * NS], F32)
        nc.sync.dma_start(out=wf[:], in_=wv[:])
        wb = cpool.tile([128, NCH * NS], BF16)
        nc.vector.tensor_copy(wb[:], wf[:])
        cf = cpool.tile([128, NCH * NS], F32)
        nc.sync.dma_start(out=cf[:], in_=cv[:])

        for blk in range(NBLK):
            bs = blk * BLK
            pairs = []
            for c in range(NCH):
                xtile = xp.tile([128, BLK], F32, tag="x")
                nc.sync.dma_start(out=xtile[:],
                                  in_=xt[c * 128:(c + 1) * 128, bs:bs + BLK])
                uf = fp.tile([128, BLK], F32, tag="uf")
                nc.scalar.activation(uf[:], xtile[:], AF.Tanh)
                q = fp.tile([128, BLK], BF16, tag="q")
                nc.scalar.activation(q[:], uf[:], AF.Square)
                t = fp.tile([128, BLK], BF16, tag="t")
                nc.vector.tensor_scalar(t[:], q[:], 2.0, -1.0, OP.mult, OP.add)
                s = fp.tile([128, BLK], BF16, tag="s")
                nc.scalar.activation(s[:], t[:], AF.Square)
                us = fp.tile([128, BLK], BF16, tag="us")
                nc.gpsimd.tensor_tensor(us[:], uf[:], s[:], OP.mult)
                e1 = fp.tile([128, BLK], BF16, tag="e1")
                nc.vector.scalar_tensor_tensor(
                    e1[:], s[:], cf[:, c * NS + 0:c * NS + 1], t[:],
                    OP.add, OP.mult)
                e2 = fp.tile([128, BLK], BF16, tag="e2")
                nc.vector.scalar_tensor_tensor(
                    e2[:], s[:], cf[:, c * NS + 1:c * NS + 2], s[:],
                    OP.add, OP.mult)
                z1 = fp.tile([128, BLK], BF16, tag="z1")
                nc.vector.scalar_tensor_tensor(
                    z1[:], t[:], cf[:, c * NS + 2:c * NS + 3], uf[:],
                    OP.add, OP.mult)
                z2 = fp.tile([128, BLK], BF16, tag="z2")
                nc.vector.scalar_tensor_tensor(
                    z2[:], t[:], cf[:, c * NS + 3:c * NS + 4], us[:],
                    OP.add, OP.mult)
                pairs += [(e1, c * NS + 0), (e2, c * NS + 1),
                          (z1, c * NS + 2), (z2, c * NS + 3)]
            NG = BLK // 512
            psA = pp.tile([64, 512], F32, tag="psA")
            psB = pp.tile([64, 512], F32, tag="psB")
            for n, (ft, col) in enumerate(pairs):
                for j in range(NG):
                    bank = (psA, psB)[j // 2]
                    row = 32 * (j % 2)
                    nc.tensor.matmul(
                        bank[row:row + 1, :], wb[:, col:col + 1],
                        ft[:, j * 512:(j + 1) * 512],
                        start=(n == 0), stop=(n == len(pairs) - 1))
            resA = fp.tile([64, 512], F32, tag="resA")
            resB = fp.tile([64, 512], F32, tag="resB")
            nc.scalar.copy(resA[:], psA[:])
            nc.vector.tensor_copy(resB[:], psB[:])
            for j in range(NG):
                rt = (resA, resB)[j // 2]
                nc.sync.dma_start(out=y[0:1, bs + j * 512:bs + (j + 1) * 512],
                                  in_=rt[32 * (j % 2):32 * (j % 2) + 1, :])
    return nc

# ---- public entry ----------------------------------------------------------
def kernel(x, coeffs, hweights, _trace=False):
    _install_patch()
    import ml_dtypes
    x = np.asarray(x, dtype=np.float32)
    W = (coeffs.astype(np.float64).T @ hweights.astype(np.float64)).reshape(D, DEG1)
    a0 = W[:, 0] - W[:, 4] + W[:, 8]
    a1 = W[:, 2] - 3 * W[:, 6]
    a2 = 2 * W[:, 4] - 8 * W[:, 8]
    a3 = 4 * W[:, 6]
    a4 = 8 * W[:, 8]
    b0 = W[:, 1] - W[:, 3] - W[:, 5] + W[:, 7]
    b1 = 2 * W[:, 3] - 2 * W[:, 5] - 4 * W[:, 7]
    b2 = 4 * W[:, 5] - 4 * W[:, 7]
    b3 = 8 * W[:, 7]
    c0 = float(a0.sum())

    def qpair(lead, partner):
        # stream weight = bf16(lead); partner rides exactly via f32 ratio
        wq = lead.astype(ml_dtypes.bfloat16).astype(np.float64)
        tiny = np.abs(wq) < 1e-30
        wq = np.where(tiny, 1e-8, wq)
        return wq.astype(np.float32), (partner / wq).astype(np.float32)

    w1, c1 = qpair(a3, a1)   # E1 = (s + a1/a3)*t * a3
    w2, c2 = qpair(a4, a2)   # E2 = (s + a2/a4)*s * a4
    w3, c3 = qpair(b1, b0)   # Z1 = (t + b0/b1)*u * b1
    w4, c4 = qpair(b3, b2)   # Z2 = (t + b2/b3)*us * b3

    wvv = np.zeros((128, NCH * NS), dtype=np.float32)
    cvv = np.zeros((128, NCH * NS), dtype=np.float32)
    for c in range(NCH):
        sl = slice(c * 128, (c + 1) * 128)
        for i, (wa, ca) in enumerate([(w1, c1), (w2, c2), (w3, c3), (w4, c4)]):
            wvv[:, c * NS + i] = wa[sl]
            cvv[:, c * NS + i] = ca[sl]

    nc = _build()
    xT = np.ascontiguousarray(x.T)                                   # [D, B]
    in_maps = [{"xt": np.ascontiguousarray(xT[:, i * BC:(i + 1) * BC]),
                "wv": wvv, "cv": cvv} for i in range(NCORES)]
    tdir = None
    if _trace:
        import tempfile
        tdir = tempfile.mkdtemp(prefix="ktrace_", dir="/tmp")
    res = run_bass_kernel_spmd(nc, in_maps, core_ids=list(range(NCORES)),
                               trace=_trace, tmpdir=tdir)
    out = np.concatenate([res.results[i]["y"][0] for i in range(NCORES)])
    if _trace:
        kernel._last = res
    return (out + np.float32(c0)).astype(np.float32)


# revision 18
# speedup vs baseline: 1.1272x; 1.1272x over previous
"""KAN layer (Chebyshev deg-8) Trainium2 kernel, 8-core data-parallel.

Math: out[b] = sum_n hw[n] * (X @ C.T)[b,n] = X[b,:] @ (C.T @ hw)
            = sum_d g_d(tanh x[d,b]),   g_d = sum_k W[d,k] T_k(u)

Reparametrize per-dim in t = T2(u) = 2u^2-1, s = t^2 (both in [-1,1], well
conditioned in bf16):
  g = a0 + a1 t + a2 s + a3 st + a4 s^2 + u(b0 + b1 t + b2 s + b3 st)
Pair the 8 non-constant coefficients into 4 streams via the ratio trick,
each one fused scalar_tensor_tensor op  w*(in0 + c)*in1:
  E1 = (s + a1/a3)*t  (w=a3)   E2 = (s + a2/a4)*s  (w=a4)
  Z1 = (t + b0/b1)*u  (w=b1)   Z2 = (t + b2/b3)*us (w=b3)
Engine split per [128,BLK] tile: ACT{tanh->f32, q=Sq(u), s=Sq(t)},
DVE{t=2q-1, E1, E2, Z1, Z2}, Pool{us=u*s}, PE{4 streams -> PSUM},
DMA{x in, PSUM -> y out}.  a0-sum is added on host.
"""
import sys
import numpy as np

sys.path.insert(0, "/opt/trn_rl_repo")

import orjson
from contextlib import ExitStack

import concourse.bass as bass
from concourse import mybir
from concourse.tile import TileContext
from concourse.bass_utils import run_bass_kernel_spmd

F32 = mybir.dt.float32
BF16 = mybir.dt.bfloat16
AF = mybir.ActivationFunctionType
OP = mybir.AluOpType

B, D, DEG1 = 32768, 256, 9
NCORES = 8
BC = B // NCORES          # 4096 batch per core
NCH = D // 128            # 2 partition chunks of dims
NS = 5                    # streams per chunk
BLK = 2048                # free-dim block for pipelining
NBLK = BC // BLK

# ---- walrus workaround: split >1 sem-waits onto Drain carriers -------------
_MAXW = 1

def _split_waits(bir_json: bytes) -> bytes:
    d = orjson.loads(bir_json)
    for fn in d.get("functions", []):
        for bb in fn.get("blocks", []):
            out = []
            for ins in bb.get("instructions", []):
                si = ins.get("sync_info") or {}
                waits = si.get("on_wait") or []
                if len(waits) > _MAXW:
                    extra, keep = waits[:-_MAXW], waits[-_MAXW:]
                    for i in range(0, len(extra), _MAXW):
                        out.append({
                            "debug": ins.get("debug", 0),
                            "engine": ins["engine"], "ins": [], "outs": [],
                            "name": f"{ins['name']}_ws{i}", "opcode": "Drain",
                            "sync_info": {"on_update": [],
                                          "on_wait": extra[i:i + _MAXW]},
                        })
                    si["on_wait"] = keep
                out.append(ins)
            bb["instructions"] = out
    return orjson.dumps(d)

def _install_patch():
    import concourse.bass_utils as bu
    if getattr(bu, "_ws_patched", False):
        return
    orig = bu.compile_bir_kernel
    def patched(bir_json, tmpdir, neff_name="file.neff"):
        return orig(_split_waits(bir_json), tmpdir, neff_name)
    bu.compile_bir_kernel = patched
    bu._ws_patched = True
    try:
        import concourse.bass2jax as b2j
        if getattr(b2j, "compile_bir_kernel", None) is orig:
            b2j.compile_bir_kernel = patched
    except Exception:
        pass

# ---- device kernel ---------------------------------------------------------
def _build():
    nc = bass.Bass()
    xt = nc.declare_dram_parameter("xt", [D, BC], F32, isOutput=False)
    wv = nc.declare_dram_parameter("wv", [128, NCH * NS], F32, isOutput=False)
    cv = nc.declare_dram_parameter("cv", [128, NCH * 4], F32, isOutput=False)
    dv = nc.declare_dram_parameter("dv", [128, NCH * 2], F32, isOutput=False)
    y = nc.declare_dram_parameter("y", [1, BC], F32, isOutput=True)

    with TileContext(nc) as tc, ExitStack() as ctx:
        cpool = ctx.enter_context(tc.tile_pool(name="const", bufs=1))
        xp = ctx.enter_context(tc.tile_pool(name="xin", bufs=3))
        fp = ctx.enter_context(tc.tile_pool(name="feat", bufs=3))
        pp = ctx.enter_context(tc.tile_pool(name="ps", bufs=2, space="PSUM"))

        wf = cpool.tile([128, NCH * NS], F32)
        nc.sync.dma_start(out=wf[:], in_=wv[:])
        wb = cpool.tile([128, NCH * NS], BF16)
        nc.vector.tensor_copy(wb[:], wf[:])
        cf = cpool.tile([128, NCH * 4], F32)
        nc.sync.dma_start(out=cf[:], in_=cv[:])
        df = cpool.tile([128, NCH * 2], F32)
        nc.sync.dma_start(out=df[:], in_=dv[:])

        for blk in range(NBLK):
            bs = blk * BLK
            pairs = []
            for c in range(NCH):
                xtile = xp.tile([128, BLK], F32, tag="x")
                nc.sync.dma_start(out=xtile[:],
                                  in_=xt[c * 128:(c + 1) * 128, bs:bs + BLK])
                uf = fp.tile([128, BLK], F32, tag="uf")
                nc.scalar.activation(uf[:], xtile[:], AF.Tanh)
                q = fp.tile([128, BLK], BF16, tag="q")
                nc.scalar.activation(q[:], uf[:], AF.Square)
                t = fp.tile([128, BLK], BF16, tag="t")
                nc.vector.tensor_scalar(t[:], q[:], 2.0, -1.0, OP.mult, OP.add)
                s = fp.tile([128, BLK], BF16, tag="s")
                nc.scalar.activation(s[:], t[:], AF.Square)
                s8 = fp.tile([128, BLK], BF16, tag="s8")
                nc.scalar.activation(s8[:], s[:], AF.Square)
                ub = fp.tile([128, BLK], BF16, tag="ub")
                nc.vector.tensor_copy(ub[:], uf[:])
                us = fp.tile([128, BLK], BF16, tag="us")
                nc.gpsimd.tensor_tensor(us[:], ub[:], s[:], OP.mult)
                a1 = fp.tile([128, BLK], BF16, tag="a1")
                nc.vector.tensor_scalar(a1[:], s[:],
                                        cf[:, c * 4 + 0:c * 4 + 1],
                                        cf[:, c * 4 + 1:c * 4 + 2],
                                        OP.mult, OP.add)
                e1 = fp.tile([128, BLK], BF16, tag="e1")
                nc.vector.tensor_tensor(e1[:], a1[:], t[:], OP.mult)
                a3 = fp.tile([128, BLK], BF16, tag="a3")
                nc.vector.tensor_scalar(a3[:], t[:],
                                        cf[:, c * 4 + 2:c * 4 + 3],
                                        cf[:, c * 4 + 3:c * 4 + 4],
                                        OP.mult, OP.add)
                z1 = fp.tile([128, BLK], BF16, tag="z1")
                nc.vector.tensor_tensor(z1[:], a3[:], ub[:], OP.mult)
                a4 = fp.tile([128, BLK], BF16, tag="a4")
                nc.vector.tensor_scalar(a4[:], t[:],
                                        df[:, c * 2 + 0:c * 2 + 1],
                                        df[:, c * 2 + 1:c * 2 + 2],
                                        OP.mult, OP.add)
                z2 = fp.tile([128, BLK], BF16, tag="z2")
                nc.vector.tensor_tensor(z2[:], a4[:], us[:], OP.mult)
                pairs += [(e1, c * NS + 0), (s, c * NS + 1),
                          (s8, c * NS + 2), (z1, c * NS + 3),
                          (z2, c * NS + 4)]
            NG = BLK // 512
            psA = pp.tile([64, 512], F32, tag="psA")
            psB = pp.tile([64, 512], F32, tag="psB")
            for n, (ft, col) in enumerate(pairs):
                for j in range(NG):
                    bank = (psA, psB)[j // 2]
                    row = 32 * (j % 2)
                    nc.tensor.matmul(
                        bank[row:row + 1, :], wb[:, col:col + 1],
                        ft[:, j * 512:(j + 1) * 512],
                        start=(n == 0), stop=(n == len(pairs) - 1))
            resA = fp.tile([64, 512], F32, tag="resA")
            resB = fp.tile([64, 512], F32, tag="resB")
            nc.scalar.copy(resA[:], psA[:])
            nc.vector.tensor_copy(resB[:], psB[:])
            for j in range(NG):
                rt = (resA, resB)[j // 2]
                nc.sync.dma_start(out=y[0:1, bs + j * 512:bs + (j + 1) * 512],
                                  in_=rt[32 * (j % 2):32 * (j % 2) + 1, :])
    return nc

# ---- public entry ----------------------------------------------------------
def kernel(x, coeffs, hweights, _trace=False):
    _install_patch()
    import ml_dtypes
    x = np.asarray(x, dtype=np.float32)
    W = (coeffs.astype(np.float64).T @ hweights.astype(np.float64)).reshape(D, DEG1)
    a0 = W[:, 0] - W[:, 4] + W[:, 8]
    a1 = W[:, 2] - 3 * W[:, 6]
    a2 = 2 * W[:, 4] - 8 * W[:, 8]
    a3 = 4 * W[:, 6]
    a4 = 8 * W[:, 8]
    b0 = W[:, 1] - W[:, 3] - W[:, 5] + W[:, 7]
    b1 = 2 * W[:, 3] - 2 * W[:, 5] - 4 * W[:, 7]
    b2 = 4 * W[:, 5] - 4 * W[:, 7]
    b3 = 8 * W[:, 7]
    c0 = float(a0.sum())

    # streams per chunk: E1=(a3*s+a1)*t [w=1], s [w=a2], s8 [w=a4],
    # Z1=(b1*t+b0)*u [w=1], Z2=(b3*t+b2)*us [w=1].  Affine coeffs ride f32
    # TS scalars (exact); only a2/a4 stream weights quantize to bf16.
    ones = np.ones(D, dtype=np.float64)
    wvv = np.zeros((128, NCH * NS), dtype=np.float32)
    cvv = np.zeros((128, NCH * 4), dtype=np.float32)
    dvv = np.zeros((128, NCH * 2), dtype=np.float32)
    for c in range(NCH):
        sl = slice(c * 128, (c + 1) * 128)
        for i, wa in enumerate([ones, a2, a4, ones, ones]):
            wvv[:, c * NS + i] = wa[sl].astype(np.float32)
        for i, ca in enumerate([a3, a1, b1, b0]):
            cvv[:, c * 4 + i] = ca[sl].astype(np.float32)
        for i, ca in enumerate([b3, b2]):
            dvv[:, c * 2 + i] = ca[sl].astype(np.float32)

    nc = _build()
    xT = np.ascontiguousarray(x.T)                                   # [D, B]
    in_maps = [{"xt": np.ascontiguousarray(xT[:, i * BC:(i + 1) * BC]),
                "wv": wvv, "cv": cvv, "dv": dvv} for i in range(NCORES)]
    tdir = None
    if _trace:
        import tempfile
        tdir = tempfile.mkdtemp(prefix="ktrace_", dir="/tmp")
    res = run_bass_kernel_spmd(nc, in_maps, core_ids=list(range(NCORES)),
                               trace=_trace, tmpdir=tdir)
    out = np.concatenate([res.results[i]["y"][0] for i in range(NCORES)])
    if _trace:
        kernel._last = res
    return (out + np.float32(c0)).astype(np.float32)


# revision 21
# speedup vs baseline: 1.1658x; 1.0342x over previous
"""KAN layer (Chebyshev deg-8) Trainium2 kernel, 8-core data-parallel.

Math: out[b] = sum_n hw[n] * (X @ C.T)[b,n] = X[b,:] @ (C.T @ hw)
            = sum_d g_d(tanh x[d,b]),   g_d = sum_k W[d,k] T_k(u)

Reparametrize per-dim in t = T2(u) = 2u^2-1, s = t^2 (both in [-1,1], well
conditioned in bf16):
  g = a0 + a1 t + a2 s + a3 st + a4 s^2 + u(b0 + b1 t + b2 s + b3 st)
Pair the 8 non-constant coefficients into 4 streams via the ratio trick,
each one fused scalar_tensor_tensor op  w*(in0 + c)*in1:
  E1 = (s + a1/a3)*t  (w=a3)   E2 = (s + a2/a4)*s  (w=a4)
  Z1 = (t + b0/b1)*u  (w=b1)   Z2 = (t + b2/b3)*us (w=b3)
Engine split per [128,BLK] tile: ACT{tanh->f32, q=Sq(u), s=Sq(t)},
DVE{t=2q-1, E1, E2, Z1, Z2}, Pool{us=u*s}, PE{4 streams -> PSUM},
DMA{x in, PSUM -> y out}.  a0-sum is added on host.
"""
import sys
import numpy as np

sys.path.insert(0, "/opt/trn_rl_repo")

import orjson
from contextlib import ExitStack

import concourse.bass as bass
from concourse import mybir
from concourse.tile import TileContext
from concourse.bass_utils import run_bass_kernel_spmd

F32 = mybir.dt.float32
BF16 = mybir.dt.bfloat16
AF = mybir.ActivationFunctionType
OP = mybir.AluOpType

B, D, DEG1 = 32768, 256, 9
NCORES = 8
BC = B // NCORES          # 4096 batch per core
NCH = D // 128            # 2 partition chunks of dims
NS = 5                    # streams per chunk
BLK = 2048                # free-dim block for pipelining
NBLK = BC // BLK

# ---- walrus workaround: split >1 sem-waits onto Drain carriers -------------
_MAXW = 1

def _split_waits(bir_json: bytes) -> bytes:
    d = orjson.loads(bir_json)
    for fn in d.get("functions", []):
        for bb in fn.get("blocks", []):
            out = []
            for ins in bb.get("instructions", []):
                si = ins.get("sync_info") or {}
                waits = si.get("on_wait") or []
                if len(waits) > _MAXW:
                    extra, keep = waits[:-_MAXW], waits[-_MAXW:]
                    for i in range(0, len(extra), _MAXW):
                        out.append({
                            "debug": ins.get("debug", 0),
                            "engine": ins["engine"], "ins": [], "outs": [],
                            "name": f"{ins['name']}_ws{i}", "opcode": "Drain",
                            "sync_info": {"on_update": [],
                                          "on_wait": extra[i:i + _MAXW]},
                        })
                    si["on_wait"] = keep
                out.append(ins)
            bb["instructions"] = out
    return orjson.dumps(d)

def _install_patch():
    import concourse.bass_utils as bu
    if getattr(bu, "_ws_patched", False):
        return
    orig = bu.compile_bir_kernel
    def patched(bir_json, tmpdir, neff_name="file.neff"):
        return orig(_split_waits(bir_json), tmpdir, neff_name)
    bu.compile_bir_kernel = patched
    bu._ws_patched = True
    try:
        import concourse.bass2jax as b2j
        if getattr(b2j, "compile_bir_kernel", None) is orig:
            b2j.compile_bir_kernel = patched
    except Exception:
        pass

# ---- device kernel ---------------------------------------------------------
def _build():
    nc = bass.Bass()
    xt = nc.declare_dram_parameter("xt", [D, BC], F32, isOutput=False)
    wv = nc.declare_dram_parameter("wv", [128, NCH * NS], F32, isOutput=False)
    cv = nc.declare_dram_parameter("cv", [128, NCH * 4], F32, isOutput=False)
    dv = nc.declare_dram_parameter("dv", [128, NCH * 2], F32, isOutput=False)
    y = nc.declare_dram_parameter("y", [1, BC], F32, isOutput=True)

    with TileContext(nc) as tc, ExitStack() as ctx:
        cpool = ctx.enter_context(tc.tile_pool(name="const", bufs=1))
        xp = ctx.enter_context(tc.tile_pool(name="xin", bufs=3))
        fp = ctx.enter_context(tc.tile_pool(name="feat", bufs=3))
        pp = ctx.enter_context(tc.tile_pool(name="ps", bufs=2, space="PSUM"))

        wf = cpool.tile([128, NCH * NS], F32)
        nc.sync.dma_start(out=wf[:], in_=wv[:])
        wb = cpool.tile([128, NCH * NS], BF16)
        nc.vector.tensor_copy(wb[:], wf[:])
        cf = cpool.tile([128, NCH * 4], F32)
        nc.sync.dma_start(out=cf[:], in_=cv[:])
        df = cpool.tile([128, NCH * 2], F32)
        nc.sync.dma_start(out=df[:], in_=dv[:])

        for blk in range(NBLK):
            bs = blk * BLK
            pairs = []
            for c in range(NCH):
                xtile = xp.tile([128, BLK], F32, tag="x")
                nc.sync.dma_start(out=xtile[:],
                                  in_=xt[c * 128:(c + 1) * 128, bs:bs + BLK])
                ub = fp.tile([128, BLK], BF16, tag="ub")
                nc.scalar.activation(ub[:], xtile[:], AF.Tanh)
                q = fp.tile([128, BLK], BF16, tag="q")
                nc.scalar.activation(q[:], ub[:], AF.Square)
                t = fp.tile([128, BLK], BF16, tag="t")
                nc.vector.tensor_scalar(t[:], q[:], 2.0, -1.0, OP.mult, OP.add)
                s = fp.tile([128, BLK], BF16, tag="s")
                nc.scalar.activation(s[:], t[:], AF.Square)
                s8 = fp.tile([128, BLK], BF16, tag="s8")
                nc.scalar.activation(s8[:], s[:], AF.Square)
                us = fp.tile([128, BLK], BF16, tag="us")
                nc.gpsimd.tensor_tensor(us[:], ub[:], s[:], OP.mult)
                a1 = fp.tile([128, BLK], BF16, tag="a1")
                nc.vector.tensor_scalar(a1[:], s[:],
                                        cf[:, c * 4 + 0:c * 4 + 1],
                                        cf[:, c * 4 + 1:c * 4 + 2],
                                        OP.mult, OP.add)
                e1 = fp.tile([128, BLK], BF16, tag="e1")
                nc.vector.tensor_tensor(e1[:], a1[:], t[:], OP.mult)
                a3 = fp.tile([128, BLK], BF16, tag="a3")
                nc.vector.tensor_scalar(a3[:], t[:],
                                        cf[:, c * 4 + 2:c * 4 + 3],
                                        cf[:, c * 4 + 3:c * 4 + 4],
                                        OP.mult, OP.add)
                z1 = fp.tile([128, BLK], BF16, tag="z1")
                nc.vector.tensor_tensor(z1[:], a3[:], ub[:], OP.mult)
                a4 = fp.tile([128, BLK], BF16, tag="a4")
                nc.vector.tensor_scalar(a4[:], t[:],
                                        df[:, c * 2 + 0:c * 2 + 1],
                                        df[:, c * 2 + 1:c * 2 + 2],
                                        OP.mult, OP.add)
                z2 = fp.tile([128, BLK], BF16, tag="z2")
                nc.vector.tensor_tensor(z2[:], a4[:], us[:], OP.mult)
                pairs += [(e1, c * NS + 0), (s, c * NS + 1),
                          (s8, c * NS + 2), (z1, c * NS + 3),
                          (z2, c * NS + 4)]
            NG = BLK // 512
            psA = pp.tile([64, 512], F32, tag="psA")
            psB = pp.tile([64, 512], F32, tag="psB")
            for j in range(NG):
                bank = (psA, psB)[j // 2]
                row = 32 * (j % 2)
                for n, (ft, col) in enumerate(pairs):
                    nc.tensor.matmul(
                        bank[row:row + 1, :], wb[:, col:col + 1],
                        ft[:, j * 512:(j + 1) * 512],
                        start=(n == 0), stop=(n == len(pairs) - 1))
            resA = fp.tile([64, 512], F32, tag="resA")
            resB = fp.tile([64, 512], F32, tag="resB")
            nc.vector.tensor_copy(resA[:], psA[:])
            nc.vector.tensor_copy(resB[:], psB[:])
            for j in range(NG):
                rt = (resA, resB)[j // 2]
                nc.sync.dma_start(out=y[0:1, bs + j * 512:bs + (j + 1) * 512],
                                  in_=rt[32 * (j % 2):32 * (j % 2) + 1, :])
    return nc

# ---- public entry ----------------------------------------------------------
def kernel(x, coeffs, hweights, _trace=False):
    _install_patch()
    import ml_dtypes
    x = np.asarray(x, dtype=np.float32)
    W = (coeffs.astype(np.float64).T @ hweights.astype(np.float64)).reshape(D, DEG1)
    a0 = W[:, 0] - W[:, 4] + W[:, 8]
    a1 = W[:, 2] - 3 * W[:, 6]
    a2 = 2 * W[:, 4] - 8 * W[:, 8]
    a3 = 4 * W[:, 6]
    a4 = 8 * W[:, 8]
    b0 = W[:, 1] - W[:, 3] - W[:, 5] + W[:, 7]
    b1 = 2 * W[:, 3] - 2 * W[:, 5] - 4 * W[:, 7]
    b2 = 4 * W[:, 5] - 4 * W[:, 7]
    b3 = 8 * W[:, 7]
    c0 = float(a0.sum())

    # streams per chunk: E1=(a3*s+a1)*t [w=1], s [w=a2], s8 [w=a4],
    # Z1=(b1*t+b0)*u [w=1], Z2=(b3*t+b2)*us [w=1].  Affine coeffs ride f32
    # TS scalars (exact); only a2/a4 stream weights quantize to bf16.
    ones = np.ones(D, dtype=np.float64)
    wvv = np.zeros((128, NCH * NS), dtype=np.float32)
    cvv = np.zeros((128, NCH * 4), dtype=np.float32)
    dvv = np.zeros((128, NCH * 2), dtype=np.float32)
    for c in range(NCH):
        sl = slice(c * 128, (c + 1) * 128)
        for i, wa in enumerate([ones, a2, a4, ones, ones]):
            wvv[:, c * NS + i] = wa[sl].astype(np.float32)
        for i, ca in enumerate([a3, a1, b1, b0]):
            cvv[:, c * 4 + i] = ca[sl].astype(np.float32)
        for i, ca in enumerate([b3, b2]):
            dvv[:, c * 2 + i] = ca[sl].astype(np.float32)

    nc = _build()
    xT = np.ascontiguousarray(x.T)                                   # [D, B]
    in_maps = [{"xt": np.ascontiguousarray(xT[:, i * BC:(i + 1) * BC]),
                "wv": wvv, "cv": cvv, "dv": dvv} for i in range(NCORES)]
    tdir = None
    if _trace:
        import tempfile
        tdir = tempfile.mkdtemp(prefix="ktrace_", dir="/tmp")
    res = run_bass_kernel_spmd(nc, in_maps, core_ids=list(range(NCORES)),
                               trace=_trace, tmpdir=tdir)
    out = np.concatenate([res.results[i]["y"][0] for i in range(NCORES)])
    if _trace:
        kernel._last = res
    return (out + np.float32(c0)).astype(np.float32)


# revision 26
# speedup vs baseline: 1.1668x; 1.0009x over previous
"""KAN layer (Chebyshev deg-8) Trainium2 kernel, 8-core data-parallel.

Math: out[b] = sum_n hw[n] * (X @ C.T)[b,n] = X[b,:] @ (C.T @ hw)
            = sum_d g_d(tanh x[d,b]),   g_d = sum_k W[d,k] T_k(u)

Reparametrize per-dim in t = T2(u) = 2u^2-1, s = t^2 (both in [-1,1], well
conditioned in bf16):
  g = a0 + a1 t + a2 s + a3 st + a4 s^2 + u(b0 + b1 t + b2 s + b3 st)
Pair the 8 non-constant coefficients into 4 streams via the ratio trick,
each one fused scalar_tensor_tensor op  w*(in0 + c)*in1:
  E1 = (s + a1/a3)*t  (w=a3)   E2 = (s + a2/a4)*s  (w=a4)
  Z1 = (t + b0/b1)*u  (w=b1)   Z2 = (t + b2/b3)*us (w=b3)
Engine split per [128,BLK] tile: ACT{tanh->f32, q=Sq(u), s=Sq(t)},
DVE{t=2q-1, E1, E2, Z1, Z2}, Pool{us=u*s}, PE{4 streams -> PSUM},
DMA{x in, PSUM -> y out}.  a0-sum is added on host.
"""
import sys
import numpy as np

sys.path.insert(0, "/opt/trn_rl_repo")

import orjson
from contextlib import ExitStack

import concourse.bass as bass
from concourse import mybir
from concourse.tile import TileContext
from concourse.bass_utils import run_bass_kernel_spmd

F32 = mybir.dt.float32
BF16 = mybir.dt.bfloat16
AF = mybir.ActivationFunctionType
OP = mybir.AluOpType

B, D, DEG1 = 32768, 256, 9
NCORES = 8
BC = B // NCORES          # 4096 batch per core
NCH = D // 128            # 2 partition chunks of dims
NS = 5                    # streams per chunk
BLK = 2048                # free-dim block for pipelining
NBLK = BC // BLK

# ---- walrus workaround: split >1 sem-waits onto Drain carriers -------------
_MAXW = 1

def _split_waits(bir_json: bytes) -> bytes:
    d = orjson.loads(bir_json)
    for fn in d.get("functions", []):
        for bb in fn.get("blocks", []):
            out = []
            for ins in bb.get("instructions", []):
                si = ins.get("sync_info") or {}
                waits = si.get("on_wait") or []
                if len(waits) > _MAXW:
                    extra, keep = waits[:-_MAXW], waits[-_MAXW:]
                    for i in range(0, len(extra), _MAXW):
                        out.append({
                            "debug": ins.get("debug", 0),
                            "engine": ins["engine"], "ins": [], "outs": [],
                            "name": f"{ins['name']}_ws{i}", "opcode": "Drain",
                            "sync_info": {"on_update": [],
                                          "on_wait": extra[i:i + _MAXW]},
                        })
                    si["on_wait"] = keep
                out.append(ins)
            bb["instructions"] = out
    return orjson.dumps(d)

def _install_patch():
    import concourse.bass_utils as bu
    if getattr(bu, "_ws_patched", False):
        return
    orig = bu.compile_bir_kernel
    def patched(bir_json, tmpdir, neff_name="file.neff"):
        return orig(_split_waits(bir_json), tmpdir, neff_name)
    bu.compile_bir_kernel = patched
    bu._ws_patched = True
    try:
        import concourse.bass2jax as b2j
        if getattr(b2j, "compile_bir_kernel", None) is orig:
            b2j.compile_bir_kernel = patched
    except Exception:
        pass

# ---- device kernel ---------------------------------------------------------
def _build():
    nc = bass.Bass()
    xt = nc.declare_dram_parameter("xt", [D, BC], F32, isOutput=False)
    wv = nc.declare_dram_parameter("wv", [128, NCH * NS], F32, isOutput=False)
    cv = nc.declare_dram_parameter("cv", [128, NCH * 4], F32, isOutput=False)
    dv = nc.declare_dram_parameter("dv", [128, NCH * 2], F32, isOutput=False)
    y = nc.declare_dram_parameter("y", [1, BC], F32, isOutput=True)

    with TileContext(nc) as tc, ExitStack() as ctx:
        cpool = ctx.enter_context(tc.tile_pool(name="const", bufs=1))
        xp = ctx.enter_context(tc.tile_pool(name="xin", bufs=3))
        fp = ctx.enter_context(tc.tile_pool(name="feat", bufs=3))
        pp = ctx.enter_context(tc.tile_pool(name="ps", bufs=2, space="PSUM"))

        wf = cpool.tile([128, NCH * NS], F32)
        nc.sync.dma_start(out=wf[:], in_=wv[:])
        wb = cpool.tile([128, NCH * NS], BF16)
        nc.vector.tensor_copy(wb[:], wf[:])
        cf = cpool.tile([128, NCH * 4], F32)
        nc.sync.dma_start(out=cf[:], in_=cv[:])
        df = cpool.tile([128, NCH * 2], F32)
        nc.sync.dma_start(out=df[:], in_=dv[:])
        negone = cpool.tile([128, 1], F32)
        nc.vector.memset(negone[:], -1.0)

        for blk in range(NBLK):
            bs = blk * BLK
            # stage-major emission across the two chunks: keeps every
            # engine fed with an independent op while the sibling chunk's
            # dependent op is still in flight.
            xtl, ubl, ql, tl, sl, s8l, utl = [], [], [], [], [], [], []
            for c in range(NCH):
                xtile = xp.tile([128, BLK], F32, tag="x")
                nc.sync.dma_start(out=xtile[:],
                                  in_=xt[c * 128:(c + 1) * 128, bs:bs + BLK])
                xtl.append(xtile)
            for c in range(NCH):
                ub = fp.tile([128, BLK], BF16, tag="ub")
                nc.scalar.activation(ub[:], xtl[c][:], AF.Tanh)
                ubl.append(ub)
            for c in range(NCH):
                q = fp.tile([128, BLK], BF16, tag="q")
                nc.scalar.activation(q[:], ubl[c][:], AF.Square)
                ql.append(q)
            for c in range(NCH):
                t = fp.tile([128, BLK], BF16, tag="t")
                nc.vector.tensor_scalar(t[:], ql[c][:], 2.0, -1.0,
                                        OP.mult, OP.add)
                tl.append(t)
            for c in range(NCH):
                # s = (2q-1)^2 straight from q - off t's critical path
                s = fp.tile([128, BLK], BF16, tag="s")
                nc.scalar.activation(s[:], ql[c][:], AF.Square,
                                     bias=negone[:], scale=2.0)
                sl.append(s)
            for c in range(NCH):
                ut = fp.tile([128, BLK], BF16, tag="ut")
                nc.gpsimd.tensor_tensor(ut[:], ubl[c][:], tl[c][:], OP.mult)
                utl.append(ut)
            for c in range(NCH):
                s8 = fp.tile([128, BLK], BF16, tag="s8")
                nc.scalar.activation(s8[:], sl[c][:], AF.Square)
                s8l.append(s8)
            pairs = []
            el = {}
            for c in range(NCH):
                a1 = fp.tile([128, BLK], BF16, tag="a1")
                nc.vector.tensor_scalar(a1[:], sl[c][:],
                                        cf[:, c * 4 + 0:c * 4 + 1],
                                        cf[:, c * 4 + 1:c * 4 + 2],
                                        OP.mult, OP.add)
                el[("a1", c)] = a1
            for c in range(NCH):
                e1 = fp.tile([128, BLK], BF16, tag="e1")
                nc.vector.tensor_tensor(e1[:], el[("a1", c)][:], tl[c][:],
                                        OP.mult)
                el[("e1", c)] = e1
            for c in range(NCH):
                a3 = fp.tile([128, BLK], BF16, tag="a3")
                nc.vector.tensor_scalar(a3[:], sl[c][:],
                                        cf[:, c * 4 + 2:c * 4 + 3],
                                        cf[:, c * 4 + 3:c * 4 + 4],
                                        OP.mult, OP.add)
                el[("a3", c)] = a3
            for c in range(NCH):
                z1 = fp.tile([128, BLK], BF16, tag="z1")
                nc.vector.tensor_tensor(z1[:], el[("a3", c)][:], ubl[c][:],
                                        OP.mult)
                el[("z1", c)] = z1
            for c in range(NCH):
                a4 = fp.tile([128, BLK], BF16, tag="a4")
                nc.vector.tensor_scalar(a4[:], sl[c][:],
                                        df[:, c * 2 + 0:c * 2 + 1],
                                        df[:, c * 2 + 1:c * 2 + 2],
                                        OP.mult, OP.add)
                el[("a4", c)] = a4
            for c in range(NCH):
                z2 = fp.tile([128, BLK], BF16, tag="z2")
                nc.vector.tensor_tensor(z2[:], el[("a4", c)][:], utl[c][:],
                                        OP.mult)
                el[("z2", c)] = z2
            for c in range(NCH):
                pairs += [(el[("e1", c)], c * NS + 0), (sl[c], c * NS + 1),
                          (s8l[c], c * NS + 2), (el[("z1", c)], c * NS + 3),
                          (el[("z2", c)], c * NS + 4)]
            NG = BLK // 512
            psA = pp.tile([64, 512], F32, tag="psA")
            psB = pp.tile([64, 512], F32, tag="psB")
            for j in range(NG):
                bank = (psA, psB)[j // 2]
                row = 32 * (j % 2)
                for n, (ft, col) in enumerate(pairs):
                    nc.tensor.matmul(
                        bank[row:row + 1, :], wb[:, col:col + 1],
                        ft[:, j * 512:(j + 1) * 512],
                        start=(n == 0), stop=(n == len(pairs) - 1))
            resA = fp.tile([64, 512], F32, tag="resA")
            resB = fp.tile([64, 512], F32, tag="resB")
            nc.vector.tensor_copy(resA[:], psA[:])
            nc.vector.tensor_copy(resB[:], psB[:])
            for j in range(NG):
                rt = (resA, resB)[j // 2]
                nc.sync.dma_start(out=y[0:1, bs + j * 512:bs + (j + 1) * 512],
                                  in_=rt[32 * (j % 2):32 * (j % 2) + 1, :])
    return nc

# ---- public entry ----------------------------------------------------------
def kernel(x, coeffs, hweights, _trace=False):
    _install_patch()
    import ml_dtypes
    x = np.asarray(x, dtype=np.float32)
    W = (coeffs.astype(np.float64).T @ hweights.astype(np.float64)).reshape(D, DEG1)
    a0 = W[:, 0] - W[:, 4] + W[:, 8]
    a1 = W[:, 2] - 3 * W[:, 6]
    a2 = 2 * W[:, 4] - 8 * W[:, 8]
    a3 = 4 * W[:, 6]
    a4 = 8 * W[:, 8]
    b0 = W[:, 1] - W[:, 3] - W[:, 5] + W[:, 7]
    b1 = 2 * W[:, 3] - 2 * W[:, 5] - 4 * W[:, 7]
    b2 = 4 * W[:, 5] - 4 * W[:, 7]
    b3 = 8 * W[:, 7]
    c0 = float(a0.sum())

    # streams per chunk: E1=(a3*s+a1)*t [w=1], s [w=a2], s8 [w=a4],
    # Z1=(b1*t+b0)*u [w=1], Z2=(b3*t+b2)*us [w=1].  Affine coeffs ride f32
    # TS scalars (exact); only a2/a4 stream weights quantize to bf16.
    ones = np.ones(D, dtype=np.float64)
    wvv = np.zeros((128, NCH * NS), dtype=np.float32)
    cvv = np.zeros((128, NCH * 4), dtype=np.float32)
    dvv = np.zeros((128, NCH * 2), dtype=np.float32)
    for c in range(NCH):
        sl = slice(c * 128, (c + 1) * 128)
        for i, wa in enumerate([ones, a2, a4, ones, ones]):
            wvv[:, c * NS + i] = wa[sl].astype(np.float32)
        for i, ca in enumerate([a3, a1, b2, b0]):
            cvv[:, c * 4 + i] = ca[sl].astype(np.float32)
        for i, ca in enumerate([b3, b1]):
            dvv[:, c * 2 + i] = ca[sl].astype(np.float32)

    nc = _build()
    xT = np.ascontiguousarray(x.T)                                   # [D, B]
    in_maps = [{"xt": np.ascontiguousarray(xT[:, i * BC:(i + 1) * BC]),
                "wv": wvv, "cv": cvv, "dv": dvv} for i in range(NCORES)]
    tdir = None
    if _trace:
        import tempfile
        tdir = tempfile.mkdtemp(prefix="ktrace_", dir="/tmp")
    res = run_bass_kernel_spmd(nc, in_maps, core_ids=list(range(NCORES)),
                               trace=_trace, tmpdir=tdir)
    out = np.concatenate([res.results[i]["y"][0] for i in range(NCORES)])
    if _trace:
        kernel._last = res
    return (out + np.float32(c0)).astype(np.float32)


# revision 27
# speedup vs baseline: 1.4709x; 1.2606x over previous
"""KAN layer (Chebyshev deg-8) Trainium2 kernel, 8-core data-parallel.

Math: out[b] = sum_n hw[n] * (X @ C.T)[b,n] = X[b,:] @ (C.T @ hw)
            = sum_d sum_k W[d,k] * T_k(tanh(x[b,d])),  W[d,k]=(C.T@hw)[d*9+k]

Device evaluates a product-Chebyshev basis (bounded, well-conditioned in bf16):
  e1=u, e2=u^2, e3=T2*u, e4=T2^2, e5=T4*u, e6=T4*T2, e7=T4*(T2*u), e8=T4^2
  with T2=2u^2-1, T4=2*T2^2-1 built on ACT/DVE; the d-contraction runs on the
  PE as 8 matvec streams per 128-d chunk accumulating into PSUM.
Host: transposes x to [D, B] (layout prep), folds hweights into coeffs, and
solves the 9x9 basis transform for per-d PE weights.
"""
import sys
import numpy as np

sys.path.insert(0, "/opt/trn_rl_repo")

import orjson
from contextlib import ExitStack

import concourse.bass as bass
from concourse import mybir
from concourse.tile import TileContext
from concourse.bass_utils import run_bass_kernel_spmd

F32 = mybir.dt.float32
BF16 = mybir.dt.bfloat16
AF = mybir.ActivationFunctionType
OP = mybir.AluOpType

B, D, DEG1 = 32768, 256, 9
NCORES = 8
BC = B // NCORES          # 4096 batch per core
NCH = D // 128            # 2 partition chunks of dims
NT = 8                    # streamed basis tensors
BLK = 2048                # free-dim block for pipelining
NBLK = BC // BLK

# ---- walrus workaround: split >1 sem-waits onto Drain carriers -------------
_MAXW = 1

def _split_waits(bir_json: bytes) -> bytes:
    d = orjson.loads(bir_json)
    for fn in d.get("functions", []):
        for bb in fn.get("blocks", []):
            out = []
            for ins in bb.get("instructions", []):
                si = ins.get("sync_info") or {}
                waits = si.get("on_wait") or []
                if len(waits) > _MAXW:
                    extra, keep = waits[:-_MAXW], waits[-_MAXW:]
                    for i in range(0, len(extra), _MAXW):
                        out.append({
                            "debug": ins.get("debug", 0),
                            "engine": ins["engine"], "ins": [], "outs": [],
                            "name": f"{ins['name']}_ws{i}", "opcode": "Drain",
                            "sync_info": {"on_update": [],
                                          "on_wait": extra[i:i + _MAXW]},
                        })
                    si["on_wait"] = keep
                out.append(ins)
            bb["instructions"] = out
    return orjson.dumps(d)

def _install_patch():
    import concourse.bass_utils as bu
    if getattr(bu, "_ws_patched", False):
        return
    orig = bu.compile_bir_kernel
    def patched(bir_json, tmpdir, neff_name="file.neff"):
        return orig(_split_waits(bir_json), tmpdir, neff_name)
    bu.compile_bir_kernel = patched
    bu._ws_patched = True
    try:
        import concourse.bass2jax as b2j
        if getattr(b2j, "compile_bir_kernel", None) is orig:
            b2j.compile_bir_kernel = patched
    except Exception:
        pass

# ---- basis transform (host) ------------------------------------------------
def _basis_matrix():
    A = np.zeros((9, 9))
    A[0, 0] = 1.0                    # e0 = T0
    A[1, 1] = 1.0                    # e1 = T1
    A[[0, 2], 2] = 0.5               # e2 = u^2   = (T0+T2)/2
    A[[1, 3], 3] = 0.5               # e3 = T2*T1 = (T1+T3)/2
    A[[0, 4], 4] = 0.5               # e4 = T2^2  = (T0+T4)/2
    A[[3, 5], 5] = 0.5               # e5 = T4*T1 = (T3+T5)/2
    A[[2, 6], 6] = 0.5               # e6 = T4*T2 = (T2+T6)/2
    A[[1, 3, 5, 7], 7] = 0.25        # e7 = T4*T2*T1
    A[[0, 8], 8] = 0.5               # e8 = T4^2  = (T0+T8)/2
    return A

# ---- device kernel ---------------------------------------------------------
def _build(c0: float):
    nc = bass.Bass()
    xt = nc.declare_dram_parameter("xt", [D, BC], F32, isOutput=False)
    wv = nc.declare_dram_parameter("wv", [128, NCH * NT], F32, isOutput=False)
    y = nc.declare_dram_parameter("y", [1, BC], F32, isOutput=True)

    with TileContext(nc) as tc, ExitStack() as ctx:
        cpool = ctx.enter_context(tc.tile_pool(name="const", bufs=1))
        xp = ctx.enter_context(tc.tile_pool(name="xin", bufs=3))
        fp = ctx.enter_context(tc.tile_pool(name="feat", bufs=3))
        op = ctx.enter_context(tc.tile_pool(name="outp", bufs=1))
        pp = ctx.enter_context(tc.tile_pool(name="ps", bufs=8, space="PSUM"))

        cb = cpool.tile([1, 1], F32)
        nc.vector.memset(cb[:], float(c0))
        wf = cpool.tile([128, NCH * NT], F32)
        nc.sync.dma_start(out=wf[:], in_=wv[:])
        wb = cpool.tile([128, NCH * NT], BF16)
        nc.vector.tensor_copy(wb[:], wf[:])

        res = op.tile([1, BC], F32)

        for blk in range(NBLK):
            bs = blk * BLK
            feats = [[None] * NT for _ in range(NCH)]
            for c in range(NCH):
                xtile = xp.tile([128, BLK], F32, tag="x")
                nc.sync.dma_start(out=xtile[:],
                                  in_=xt[c * 128:(c + 1) * 128, bs:bs + BLK])
                uf = fp.tile([128, BLK], F32, tag="uf")
                nc.scalar.activation(uf[:], xtile[:], AF.Tanh)
                u = fp.tile([128, BLK], BF16, tag="u")
                nc.vector.tensor_copy(u[:], uf[:])
                q2 = fp.tile([128, BLK], BF16, tag="q2")
                nc.scalar.activation(q2[:], uf[:], AF.Square)
                t2 = fp.tile([128, BLK], BF16, tag="t2")
                nc.vector.tensor_scalar(t2[:], q2[:], 2.0, -1.0, OP.mult, OP.add)
                s4 = fp.tile([128, BLK], BF16, tag="s4")
                nc.scalar.activation(s4[:], t2[:], AF.Square)
                t4 = fp.tile([128, BLK], BF16, tag="t4")
                nc.vector.tensor_scalar(t4[:], s4[:], 2.0, -1.0, OP.mult, OP.add)
                p3 = fp.tile([128, BLK], BF16, tag="p3")
                nc.vector.tensor_mul(p3[:], t2[:], u[:])
                p5 = fp.tile([128, BLK], BF16, tag="p5")
                nc.vector.tensor_mul(p5[:], t4[:], u[:])
                p6 = fp.tile([128, BLK], BF16, tag="p6")
                nc.vector.tensor_mul(p6[:], t4[:], t2[:])
                p7 = fp.tile([128, BLK], BF16, tag="p7")
                nc.vector.tensor_mul(p7[:], t4[:], p3[:])
                s8 = fp.tile([128, BLK], BF16, tag="s8")
                nc.vector.tensor_mul(s8[:], t4[:], t4[:])
                feats[c] = [u, q2, p3, s4, p5, p6, p7, s8]
            for j in range(BLK // 512):
                ps = pp.tile([1, 512], F32)
                n = 0
                for c in range(NCH):
                    for t in range(NT):
                        nc.tensor.matmul(
                            ps[:], wb[:, c * NT + t:c * NT + t + 1],
                            feats[c][t][:, j * 512:(j + 1) * 512],
                            start=(n == 0), stop=(n == 2 * NT - 1))
                        n += 1
                nc.scalar.activation(res[:, bs + j * 512:bs + (j + 1) * 512],
                                     ps[:], AF.Identity, bias=cb[:])
        nc.sync.dma_start(out=y[:], in_=res[:])
    return nc

# ---- public entry ----------------------------------------------------------
def kernel(x, coeffs, hweights, _trace=False):
    _install_patch()
    x = np.asarray(x, dtype=np.float32)
    w = (coeffs.astype(np.float64).T @ hweights.astype(np.float64))  # [2304]
    W = w.reshape(D, DEG1)                                           # [d, k]
    # quantization-compensated solve: peel leading Chebyshev components in
    # decreasing degree; each tensor's bf16 weight rounding is re-absorbed by
    # the lower-degree tensors, leftover T0 lands in the exact fp32 const.
    import ml_dtypes
    A = _basis_matrix()
    Wc = W.astype(np.float64).copy()
    lam = np.zeros((D, DEG1))
    for t in range(DEG1 - 1, 0, -1):       # e8..e1, leading cheb index == t
        lt = Wc[:, t] / A[t, t]
        ltq = lt.astype(ml_dtypes.bfloat16).astype(np.float64)
        Wc -= ltq[:, None] * A[:, t][None, :]
        lam[:, t] = ltq
    c0 = float(Wc[:, 0].sum())
    wv = np.zeros((128, NCH * NT), dtype=np.float32)
    for c in range(NCH):
        for t in range(NT):
            wv[:, c * NT + t] = lam[c * 128:(c + 1) * 128, t + 1]

    nc = _build(c0)
    xT = np.ascontiguousarray(x.T)                                   # [D, B]
    in_maps = [{"xt": np.ascontiguousarray(xT[:, i * BC:(i + 1) * BC]),
                "wv": wv} for i in range(NCORES)]
    tdir = None
    if _trace:
        import tempfile
        tdir = tempfile.mkdtemp(prefix="ktrace_", dir="/tmp")
    res = run_bass_kernel_spmd(nc, in_maps, core_ids=list(range(NCORES)),
                               trace=_trace, tmpdir=tdir)
    out = np.concatenate([res.results[i]["y"][0] for i in range(NCORES)])
    if _trace:
        kernel._last = res
    return out.astype(np.float32)

